# revision 28
# baseline (speedup 1.0000x reference)
"""Trainium2 Bass kernel for a full attention block (QKV proj -> softmax(QK^T/sqrt(d)
+ rel + mask) -> ctx -> out proj -> residual -> layernorm), returning (out, attn).

Sharding: 8 cores = 4 batches x 2 sequence-halves (data parallel, zero collectives).
Each core computes all 16 heads for its 512 query rows; k/v are computed for the
full 1024-sequence of its batch (duplicated across the 2 cores of a batch -- the
kernel is HBM-bound, so the duplicate FLOPs are free).

Device-side layout is fully "transposed": scores are computed as scoresT[t, s]
tiles (t on partitions), which makes
  - the key-padding mask a per-partition ACT bias (fused into the Exp op),
  - the rel bias a PE identity-matmul accumulation directly into the scores PSUM,
  - the context matmul ctxT[c, s] += v[t, c]-slice.T @ expT[t, s] transpose-free,
  - ctxT the natural stationary operand for the output projection.
The host feeds rel pre-transposed per head ([t, s] slabs) and un-transposes the
attention-probability output when assembling the full result; that is part of the
shard/unshard glue. Softmax denominators ride the ctx matmul itself (a ones
column appended to v drops the per-(head, s) sums into PSUM row 64); the
reciprocal row is broadcast across partitions with a rank-1 PE matmul. Softmax skips max-subtraction (scores are
bounded by ~e^8, and masked lanes underflow to exactly 0 like the reference).

All matmuls run on the PE's full-rate fp32 path (dtype float32r: fp32 values
rounded to an 8-bit-exponent/11-bit-mantissa grid, 1 cycle/row instead of 4).
DMA-fed operands (x^T, weights, rel) are pre-rounded to that grid on the host;
compute-produced operands are rounded by tagging the producing ACT/DVE
instruction's output as float32r. PSUM accumulation stays full fp32, and the
residual + layernorm path is exact fp32, so the out tensor sees ~1e-5 error and
attn ~5e-4. fp32r matmuls require dst partition 0, so odd heads' context is
accumulated at partitions 0-63 and shifted to 64-127 with a plain-fp32 identity
matmul.

The q/k projections are fused into the attention loop (one 128-channel strip of
q^T/k^T feeds exactly 2 heads), so the projection PE work overlaps the
DMA-heavy softmax phase, and one shared PSUM pool with rotating tags spans all
phases (no inter-phase PSUM barrier). Timeline-simulated at ~331 us/core against
a ~260 us DMA floor (rel in + attn out are 32 MB each per core); HW-validated
errors: out 1.7e-5, attn 5.2e-4.
"""

import sys

sys.path.insert(0, "/opt/trn_rl_repo")

import numpy as np

import concourse.bass as bass
import concourse.mybir as mybir
import concourse.tile as tile
from concourse import bacc
from concourse.bass_utils import run_bass_kernel_spmd

HEAD, SIZE = 16, 64
DIM = HEAD * SIZE  # 1024
S = 1024
B = 4
NINF = -10000.0
EPS = 1e-6

P = 128
KD = DIM // P  # 8 contraction tiles over the model dim
KT = S // P  # 8 tiles over the key/value sequence
SOWN = S // 2  # query rows owned by one core
NCORES = 8
F32 = mybir.dt.float32
F32R = mybir.dt.float32r  # full-rate fp32 matmul path (1 cycle/row at N>=256)


def _r(ap):
    """Bitcast an fp32 AP to float32r for full-rate PE streaming."""
    return ap.bitcast(F32R)

LAST_RESULTS = None  # BassKernelResults of the most recent run (for test harness)


def _fbc2(ap):
    """View a [P, N] AP as [P, 2, N] with a stride-0 middle dim."""
    dims = [list(d) for d in ap.ap]
    return bass.AP(tensor=ap.tensor, offset=ap.offset, ap=[dims[0], [0, 2]] + dims[1:])


def _bcast(src_ap, parts):
    """AP that replicates a 1-row AP across `parts` partitions (partition step 0)."""
    dims = [list(d) for d in src_ap.ap]
    if len(dims) > 1 and dims[0][1] == 1:
        dims = dims[1:]
    return bass.AP(tensor=src_ap.tensor, offset=src_ap.offset, ap=[[0, parts]] + dims)


def _emit(tc):
    nc = tc.nc
    AF = mybir.ActivationFunctionType

    xT_d = nc.dram_tensor("xT", [DIM, S], F32R, kind="ExternalInput").ap()
    xres_d = nc.dram_tensor("xres", [SOWN, DIM], F32, kind="ExternalInput").ap()
    relT_d = nc.dram_tensor("relT", [HEAD, S, SOWN], F32R, kind="ExternalInput").ap()
    maskpk_d = nc.dram_tensor("maskpk", [P, KT], F32, kind="ExternalInput").ap()
    wq_d = nc.dram_tensor("wq", [DIM, DIM], F32R, kind="ExternalInput").ap()
    wk_d = nc.dram_tensor("wk", [DIM, DIM], F32R, kind="ExternalInput").ap()
    wv_d = nc.dram_tensor("wv", [DIM, DIM], F32R, kind="ExternalInput").ap()
    wo_d = nc.dram_tensor("wo", [DIM, DIM], F32R, kind="ExternalInput").ap()
    bqpk_d = nc.dram_tensor("bqpk", [P, KD], F32, kind="ExternalInput").ap()
    bkpk_d = nc.dram_tensor("bkpk", [P, KD], F32, kind="ExternalInput").ap()
    bv_d = nc.dram_tensor("bv", [DIM], F32, kind="ExternalInput").ap()
    gamma_d = nc.dram_tensor("gamma", [DIM], F32, kind="ExternalInput").ap()
    beta_d = nc.dram_tensor("beta", [DIM], F32, kind="ExternalInput").ap()

    attnT_d = nc.dram_tensor("attnT", [HEAD, S, SOWN], F32, kind="ExternalOutput").ap()
    out_d = nc.dram_tensor("out", [SOWN, DIM], F32, kind="ExternalOutput").ap()

    with (
        tc.tile_pool(name="consts", bufs=1) as consts,
        tc.tile_pool(name="wght", bufs=1) as wght_pool,
        tc.tile_pool(name="vp", bufs=1) as v_pool,
        tc.tile_pool(name="ctxp", bufs=1) as ctx_pool,
        tc.tile_pool(name="psum", bufs=2, space="PSUM") as psum,
    ):
        maskpk = consts.tile([P, KT], F32, tag="maskpk")
        nc.sync.dma_start(maskpk[:, :], maskpk_d)
        bqpk = consts.tile([P, KD], F32, tag="bqpk")
        nc.sync.dma_start(bqpk[:, :], bqpk_d)
        bkpk = consts.tile([P, KD], F32, tag="bkpk")
        nc.sync.dma_start(bkpk[:, :], bkpk_d)
        bvbc = consts.tile([P, DIM], F32, tag="bvbc")
        nc.sync.dma_start(bvbc[:, :], _bcast(bv_d, P))
        gammabc = consts.tile([P, DIM], F32, tag="gammabc")
        nc.sync.dma_start(gammabc[:, :], _bcast(gamma_d, P))
        betabc = consts.tile([P, DIM], F32, tag="betabc")
        nc.sync.dma_start(betabc[:, :], _bcast(beta_d, P))
        epst = consts.tile([P, 1], F32, tag="epst")
        nc.vector.memset(epst[:, :], EPS)
        from concourse.masks import make_identity

        identp = consts.tile([P, P], F32, tag="identp")
        make_identity(nc, identp[:, :])
        identr = consts.tile([P, P], F32, tag="identr")
        nc.scalar.activation(identr[:, :].bitcast(F32R), identp[:, :], AF.Identity, bias=0.0, scale=1.0)

        # fp32r ones (memset cannot emit fp32r, so produce through ACT).
        ones_pp = consts.tile([P, P], F32, tag="ones_pp")
        nc.scalar.activation(ones_pp[:, :].bitcast(F32R), identp[:, :], AF.Identity, bias=1.0, scale=0.0)

        vsb = v_pool.tile([P, KT, HEAD, SIZE + 1], F32, tag="vsb")  # v natural [t, c] + ones col
        nc.scalar.activation(
            vsb[:, :, :, SIZE : SIZE + 1].bitcast(F32R), identp[:, 0 : KT * HEAD], AF.Identity, bias=1.0, scale=0.0
        )
        ctxT = ctx_pool.tile([P, KD, SOWN], F32, tag="ctxT")  # normalized context^T

        attn_pools = (
            tc.tile_pool(name="xTp", bufs=1),
            tc.tile_pool(name="wqk", bufs=2),
            tc.tile_pool(name="qts", bufs=2),
            tc.tile_pool(name="kts", bufs=2),
            tc.tile_pool(name="rel", bufs=3),
            tc.tile_pool(name="expp", bufs=5),
            tc.tile_pool(name="rcp", bufs=2),
            tc.tile_pool(name="ntp", bufs=2),
        )
        import contextlib

        inner = contextlib.ExitStack()
        xT_pool, wqk_pool, qts_pool, kts_pool, rel_pool, exp_pool, rc_pool, nt_pool = (
            inner.enter_context(p) for p in attn_pools
        )

        xT = xT_pool.tile([P, KD, S], F32R, tag="xT")
        for kd in range(KD):
            nc.sync.dma_start(xT[:, kd, :], xT_d[kd * P : (kd + 1) * P, :])

        # ---------- v projection, one cb = 8 heads' channels ----------
        def emit_v_projection(cb):
            wvs = []
            for kd in range(KD):
                w = wght_pool.tile([P, 512], F32R, tag=f"w{kd}", name=f"wv{cb}_{kd}")
                nc.sync.dma_start(w[:, :], wv_d[kd * P : (kd + 1) * P, cb * 512 : (cb + 1) * 512])
                wvs.append(w)
            for kt in range(KT):
                pv = psum.tile([P, 512], F32, tag="d")
                for kd in range(KD):
                    nc.tensor.matmul(
                        pv[:, :], xT[:, kd, kt * P : (kt + 1) * P], wvs[kd][:, :], start=kd == 0, stop=kd == KD - 1
                    )
                nc.vector.tensor_add(
                    vsb[:, kt, cb * 8 : (cb + 1) * 8, 0:SIZE].bitcast(F32R),
                    pv[:, :].rearrange("p (h c) -> p h c", c=SIZE),
                    bvbc[:, cb * 512 : (cb + 1) * 512].rearrange("p (h c) -> p h c", c=SIZE),
                )

        # cb=0 feeds heads 0-7 and must precede the loop; cb=1 (heads 8-15) is
        # deferred to kc==4 so its PE work fills gaps in the DMA-bound
        # attention phase instead of extending the PE-dense startup. The Wo
        # strips reuse the same weight-pool slots and are only needed by the
        # output projection, so they prefetch from kc==6.
        emit_v_projection(0)
        wos = []

        # ---------- fused q/k projection + attention, one kc strip = 2 heads ----------
        for kc in range(KD):
            if kc == 4:
                emit_v_projection(1)
            if kc == 6:
                for kd in range(KD):
                    w = wght_pool.tile([P, DIM], F32R, tag=f"w{kd}", name=f"wo{kd}")
                    nc.sync.dma_start(w[:, :], wo_d[kd * P : (kd + 1) * P, :])
                    wos.append(w)
            wqc = wqk_pool.tile([P, KD, P], F32R, tag="wqc")
            nc.sync.dma_start(
                wqc[:, :, :], wq_d[:, kc * P : (kc + 1) * P].rearrange("(kd p) c -> p kd c", p=P)
            )
            wkc = wqk_pool.tile([P, KD, P], F32R, tag="wkc")
            nc.sync.dma_start(
                wkc[:, :, :], wk_d[:, kc * P : (kc + 1) * P].rearrange("(kd p) c -> p kd c", p=P)
            )
            pq = psum.tile([P, SOWN], F32, tag="a")
            pkA = psum.tile([P, 512], F32, tag="b")
            pkB = psum.tile([P, 512], F32, tag="c")
            for kd in range(KD):
                st, sp = kd == 0, kd == KD - 1
                nc.tensor.matmul(pq[:, :], wqc[:, kd, :], xT[:, kd, 0:SOWN], start=st, stop=sp)
                nc.tensor.matmul(pkA[:, :], wkc[:, kd, :], xT[:, kd, 0:512], start=st, stop=sp)
                nc.tensor.matmul(pkB[:, :], wkc[:, kd, :], xT[:, kd, 512:1024], start=st, stop=sp)
            qts = qts_pool.tile([P, SOWN], F32, tag="qts")
            nc.scalar.activation(qts[:, :].bitcast(F32R), pq[:, :], AF.Identity, bias=bqpk[:, kc : kc + 1], scale=1.0)
            kts = kts_pool.tile([P, S], F32, tag="kts")
            nc.scalar.activation(kts[:, 0:512].bitcast(F32R), pkA[:, :], AF.Identity, bias=bkpk[:, kc : kc + 1], scale=1.0)
            nc.scalar.activation(kts[:, 512:1024].bitcast(F32R), pkB[:, :], AF.Identity, bias=bkpk[:, kc : kc + 1], scale=1.0)

            for h in (2 * kc, 2 * kc + 1):
                pb = SIZE * (h % 2)
                exps = []
                for kt2 in range(KT // 2):
                    rt2 = rel_pool.tile([P, 2, SOWN], F32R, tag="rt")
                    nc.sync.dma_start(
                        rt2[:, :, :],
                        relT_d[h, kt2 * 2 * P : (kt2 + 1) * 2 * P, :].rearrange("(j p) s -> p j s", p=P),
                    )
                    et2 = exp_pool.tile([P, 2, SOWN], F32, tag="et")
                    for j in range(2):
                        kt = 2 * kt2 + j
                        ps = psum.tile([P, SOWN], F32, tag="c")
                        nc.tensor.matmul(
                            ps[:, :],
                            _r(kts[pb : pb + SIZE, kt * P : (kt + 1) * P]),
                            _r(qts[pb : pb + SIZE, :]),
                            start=True,
                            stop=False,
                        )
                        # rel bias accumulates into the scores PSUM via an
                        # identity matmul (frees the DVE from 128 adds).
                        nc.tensor.matmul(ps[:, :], _r(identr[:, :]), rt2[:, j, :], start=False, stop=True)
                        nc.scalar.activation(
                            et2[:, j, :].bitcast(F32R), ps[:, :], AF.Exp, bias=maskpk[:, kt : kt + 1], scale=1.0
                        )
                    exps.append(et2)

                pc = psum.tile([P, SOWN], F32, tag="a" if pb == 0 else "b")
                for kt in range(KT):
                    ej = exps[kt // 2][:, kt % 2, :]
                    # fp32r matmuls require dst start_partition 0, so both head
                    # parities accumulate at partitions 0..63 (odd heads are
                    # shifted to 64..127 afterwards via an identity matmul);
                    # the ones column of vsb drops the softmax denominator
                    # into row 64 of the same accumulation.
                    nc.tensor.matmul(
                        pc[0 : SIZE + 1, :], _r(vsb[:, kt, h, :]), _r(ej), start=kt == 0, stop=kt == KT - 1
                    )

                rc = rc_pool.tile([P, SOWN], F32, tag="rc")
                with nc.allow_low_precision(reason="fp32r rounding of softmax reciprocal"):
                    nc.vector.reciprocal(rc[SIZE : SIZE + 1, :].bitcast(F32R), pc[SIZE : SIZE + 1, :])
                # Broadcast the reciprocal row across partitions on the PE
                # (rank-1 ones matmul), then evict to SBUF so gpsimd can read it.
                rp = psum.tile([P, SOWN], F32, tag="d")
                nc.tensor.matmul(
                    rp[:, :], _r(ones_pp[SIZE : SIZE + 1, :]), _r(rc[SIZE : SIZE + 1, :]), start=True, stop=True
                )
                rcb = rc_pool.tile([P, SOWN], F32, tag="rcb")
                nc.vector.tensor_copy(rcb[:, :], rp[:, :])

                if pb == 0:
                    nc.vector.tensor_mul(ctxT[0:SIZE, kc, :].bitcast(F32R), pc[0:SIZE, :], rcb[0:SIZE, :])
                else:
                    ctmp = rc_pool.tile([SIZE, SOWN], F32, tag="ctmp", bufs=1)
                    nc.vector.tensor_mul(ctmp[:, :], pc[0:SIZE, :], rcb[0:SIZE, :])
                    csh = psum.tile([P, SOWN], F32, tag="d")
                    nc.tensor.matmul(csh[pb : pb + SIZE, :], identp[0:SIZE, 0:SIZE], ctmp[:, :], start=True, stop=True)
                    nc.vector.tensor_copy(ctxT[pb : pb + SIZE, kc, :].bitcast(F32R), csh[pb : pb + SIZE, :])
                for kt2 in range(KT // 2):
                    et2 = exps[kt2]
                    nt2 = nt_pool.tile([P, 2, SOWN], F32, tag="nt")
                    eng = nc.gpsimd if kt2 < 2 else nc.vector
                    eng.tensor_mul(nt2[:, :, :], et2[:, :, :], _fbc2(rcb[:, :]))
                    nc.sync.dma_start(
                        attnT_d[h, kt2 * 2 * P : (kt2 + 1) * 2 * P, :].rearrange("(j p) s -> p j s", p=P),
                        nt2[:, :, :],
                    )

        inner.close()

        # ---------- output projection + residual + layernorm ----------
        with (
            tc.tile_pool(name="xrp", bufs=2) as xr_pool,
            tc.tile_pool(name="resp", bufs=2) as res_pool,
            tc.tile_pool(name="lnp", bufs=4) as ln_pool,
        ):
            for stt in range(SOWN // P):
                xr = xr_pool.tile([P, DIM], F32, tag="xr")
                nc.sync.dma_start(xr[:, :], xres_d[stt * P : (stt + 1) * P, :])
                res = res_pool.tile([P, DIM], F32, tag="res")
                for ob in range(2):
                    pr = psum.tile([P, 512], F32, tag="a")
                    for kd in range(KD):
                        nc.tensor.matmul(
                            pr[:, :],
                            _r(ctxT[:, kd, stt * P : (stt + 1) * P]),
                            wos[kd][:, ob * 512 : (ob + 1) * 512],
                            start=kd == 0,
                            stop=kd == KD - 1,
                        )
                    nc.vector.tensor_add(
                        res[:, ob * 512 : (ob + 1) * 512], pr[:, :], xr[:, ob * 512 : (ob + 1) * 512]
                    )
                stats = ln_pool.tile([P, 2, 6], F32, tag="stats")
                nc.vector.bn_stats(stats[:, 0, :], res[:, 0:512])
                nc.vector.bn_stats(stats[:, 1, :], res[:, 512:1024])
                mv = ln_pool.tile([P, 2], F32, tag="mv")
                nc.vector.bn_aggr(mv[:, :], stats[:, :, :])
                sd = ln_pool.tile([P, 1], F32, tag="sd")
                nc.scalar.activation(sd[:, :], mv[:, 1:2], AF.Sqrt, bias=epst[:, :], scale=1.0)
                rstd = ln_pool.tile([P, 1], F32, tag="rstd")
                nc.vector.reciprocal(rstd[:, :], sd[:, :])
                nc.vector.tensor_scalar(
                    out=res[:, :],
                    in0=res[:, :],
                    scalar1=mv[:, 0:1],
                    scalar2=rstd[:, :],
                    op0=mybir.AluOpType.subtract,
                    op1=mybir.AluOpType.mult,
                )
                nc.vector.tensor_mul(res[:, :], res[:, :], gammabc[:, :])
                nc.vector.tensor_add(res[:, :], res[:, :], betabc[:, :])
                nc.sync.dma_start(out_d[stt * P : (stt + 1) * P, :], res[:, :])


_CACHE = {}


def _get_program():
    if "nc" not in _CACHE:
        nc = bacc.Bacc("TRN2", target_bir_lowering=False, debug=False, num_devices=NCORES)
        with tile.TileContext(nc) as tc:
            _emit(tc)
        nc.compile()
        _CACHE["nc"] = nc
    return _CACHE["nc"]


def _perm_for(half):
    if half == 0:
        return np.arange(S)
    return np.concatenate([np.arange(SOWN, S), np.arange(0, SOWN)])


def _round_f32r(a):
    """Round fp32 values to the FP32R grid (8-bit exp, 11-bit mantissa): RNE on
    the low 12 mantissa bits, which the PE's full-rate fp32 path requires."""
    u = np.ascontiguousarray(a, dtype=np.float32).view(np.uint32)
    u = u + 0x7FF + ((u >> 12) & 1)
    u &= np.uint32(0xFFFFF000)
    return u.view(np.float32)


def _make_in_maps(x, mask, rel, Wq, bq, Wk, bk, Wv, bv, Wo, bo, gamma, beta):
    f = lambda a: np.ascontiguousarray(np.asarray(a, dtype=np.float32))
    x, rel = f(x), f(rel)
    mask = np.asarray(mask)
    WqS = _round_f32r(f(Wq) * np.float32(0.125))
    Wk_c, Wv_c, Wo_c = _round_f32r(Wk), _round_f32r(Wv), _round_f32r(Wo)
    bq_pk = np.ascontiguousarray((f(bq) * np.float32(0.125)).reshape(KD, P).T)
    bk_pk = np.ascontiguousarray(f(bk).reshape(KD, P).T)
    bv_f, gamma_f, beta_f = f(bv), f(gamma), f(beta)
    bo_f = f(bo)

    in_maps = []
    for core in range(NCORES):
        b, half = core // 2, core % 2
        sl = slice(half * SOWN, (half + 1) * SOWN)
        perm = _perm_for(half)
        xb = x[b]
        xT_in = _round_f32r(xb[perm].T)
        xres_in = np.ascontiguousarray(xb[sl] + bo_f)
        relT_in = _round_f32r(rel[0][:, sl, :][:, :, perm].transpose(0, 2, 1))
        maskN = (mask[b].astype(np.float32) * np.float32(NINF))[perm]
        maskpk_in = np.ascontiguousarray(maskN.reshape(KT, P).T)
        in_maps.append(
            dict(
                xT=xT_in,
                xres=xres_in,
                relT=relT_in,
                maskpk=maskpk_in,
                wq=WqS,
                wk=Wk_c,
                wv=Wv_c,
                wo=Wo_c,
                bqpk=bq_pk,
                bkpk=bk_pk,
                bv=bv_f,
                gamma=gamma_f,
                beta=beta_f,
            )
        )
    return in_maps


def kernel(x, mask, rel, Wq, bq, Wk, bk, Wv, bv, Wo, bo, gamma, beta):
    global LAST_RESULTS
    nc = _get_program()
    in_maps = _make_in_maps(x, mask, rel, Wq, bq, Wk, bk, Wv, bv, Wo, bo, gamma, beta)
    res = run_bass_kernel_spmd(nc, in_maps, list(range(NCORES)))
    LAST_RESULTS = res

    out = np.empty((B, S, DIM), np.float32)
    attn = np.empty((B, HEAD, S, S), np.float32)
    for core in range(NCORES):
        b, half = core // 2, core % 2
        sl = slice(half * SOWN, (half + 1) * SOWN)
        perm = _perm_for(half)
        r = res.results[core]
        out[b, sl] = r["out"]
        # r["attnT"][h, t', s] holds attn[b, h, half*SOWN + s, perm[t']]; the
        # block-swap perm is its own inverse, so Y[:, :, perm] undoes it.
        attn[b, :, sl, :] = r["attnT"].transpose(0, 2, 1)[:, :, perm]
    return out, attn
